# revision 42
# baseline (speedup 1.0000x reference)
"""DecoderRNN (LSTM + vocab projection) Trainium2 kernel.

Strategy: data-parallel over batch B=64 across 8 NeuronCores (8 examples
per core). Per core:
  1. indirect-DMA gather of caption embeddings (bf16) issued at t0,
     XBAR DMA-transpose -> X.T (no PE involvement); feature rows arrive
     pre-transposed from the host.
  2. one GEMM precomputes X @ W_ih.T + b for all 33 cell steps,
     rotating across 4 PSUM banks so the PE never stalls on drain.
  3. 33 sequential LSTM cell steps in transposed layout:
       gates.T = W_hh.T.T @ h.T accumulated into PSUM tiles split by gate
       group (i+f / g / o), each preloaded with its X-projection term via an
       identity matmul; eltwise runs as a few [128, 32..64] ACT/DVE ops.
       From step 17 on, FC chunks for the first half of the tokens are
       interleaved between steps to fill the per-step PE stall and to
       spread the output writeback across the recurrence window.
  4. remaining FC GEMM chunks + bias, bf16 output DMA (host upcasts).
All matmuls take bf16 inputs with fp32 PSUM accumulation.
"""

import numpy as np
import ml_dtypes

import concourse.bass as bass
import concourse.tile as tile
from concourse import bacc, mybir
from concourse import bass_utils
from concourse.masks import make_identity

BF16 = ml_dtypes.bfloat16

# Problem shape (hardcoded per the task contract).
B, T, E, H, V = 64, 32, 512, 512, 10000
NCORES = 8
BL = B // NCORES            # 8 examples per core
STEPS = T + 1               # 33 cell steps (features + 32 caption tokens)
FOURH = 4 * H               # 2048
P = 128
NJ = FOURH // P             # 16 gate-unit chunks
NK = H // P                 # 4 contraction chunks
TOKR = T * BL               # 256 token rows (t-major)
FPAD = 2 * BL               # features (8) + pad (8): 16-aligned prefix so
                            # XBAR transpose dest offsets stay tile-aligned
ROWS = FPAD + TOKR          # 272 X.T columns per k-chunk:
                            # [features 0:8 | pad 8:16 | tokens 16:272]
ARW = FPAD + TOKR // 2      # 144 = xproj phase-A columns (features+steps 1-16)
VP = 10240                  # padded vocab (20 * 512)
NV = VP // 512              # 20 vocab chunks

f32 = mybir.dt.float32
bf16 = mybir.dt.bfloat16
i32 = mybir.dt.int32
fp8 = mybir.dt.float8e4
FP8 = ml_dtypes.float8_e4m3

# Gate pack order along 4H is (i, f, g, o) — PyTorch's native order, so the
# chain-critical i/f sigmoids can start while the g/o matmuls still stream.
_PERM = np.arange(4 * H)


def _build_program():
    nc = bacc.Bacc(
        "TRN2",
        target_bir_lowering=False,
        debug=False,
        num_devices=NCORES,
    )

    x_featT = nc.dram_tensor("x_featT", [E, BL], bf16, kind="ExternalInput").ap()
    tok_idx = nc.dram_tensor("tok_idx", [TOKR, 1], i32, kind="ExternalInput").ap()
    embed_w = nc.dram_tensor("embed_w", [V, E], bf16, kind="ExternalInput").ap()
    w_ihT = nc.dram_tensor("w_ihT", [E, FOURH], bf16, kind="ExternalInput").ap()
    w_hhT = nc.dram_tensor("w_hhT", [H, FOURH], bf16, kind="ExternalInput").ap()
    bias_t = nc.dram_tensor("bias_t", [P, NJ], f32, kind="ExternalInput").ap()
    fc_wT = nc.dram_tensor("fc_wT", [H, VP], bf16, kind="ExternalInput").ap()
    fc_b_bc = nc.dram_tensor("fc_b_bc", [P, VP], bf16, kind="ExternalInput").ap()
    out = nc.dram_tensor("out", [BL, T, V], bf16, kind="ExternalOutput").ap()

    with tile.TileContext(nc) as tc:
        _kernel_body(tc, x_featT, tok_idx, embed_w, w_ihT, w_hhT, bias_t,
                     fc_wT, fc_b_bc, out)

    nc.compile()
    return nc


def _kernel_body(tc, x_featT, tok_idx, embed_w, w_ihT, w_hhT, bias_t,
                 fc_wT, fc_b_bc, out):
    from contextlib import ExitStack
    from concourse.tile_rust import add_dep_helper as _adh
    ctx = ExitStack()
    nc = tc.nc

    # ---- persistent tiles (one bufs=1 pool, distinct tags per name) ----
    cp = ctx.enter_context(tc.tile_pool(name="const", bufs=1))
    wih_sb = cp.tile([P, NK * FOURH], bf16, name="wih_sb", tag="wih_sb")
    whh_sb = cp.tile([P, NK * FOURH], bf16, name="whh_sb", tag="whh_sb")
    fcw_sb = cp.tile([P, NK * VP], bf16, name="fcw_sb", tag="fcw_sb")
    fcb_sb = cp.tile([P, VP], bf16, name="fcb_sb", tag="fcb_sb")
    biast_sb = cp.tile([P, NJ], f32, name="biast_sb", tag="biast_sb")
    ident = cp.tile([P, P], bf16, name="ident", tag="ident")
    idx_sb = cp.tile([P, 2], i32, name="idx_sb", tag="idx_sb")
    xn0 = cp.tile([P, E], bf16, name="xn0", tag="xn0")
    xn1 = cp.tile([P, E], bf16, name="xn1", tag="xn1")
    xT = cp.tile([P, NK * ROWS], bf16, name="xT", tag="xT")
    xpT = cp.tile([P, STEPS * P], bf16, name="xpT", tag="xpT")
    hT = cp.tile([P, NK * TOKR], bf16, name="hT", tag="hT")
    h0T = cp.tile([P, NK * BL], bf16, name="h0T", tag="h0T")
    cst = cp.tile([P, NK * BL], f32, name="cst", tag="cst")

    ps = ctx.enter_context(tc.tile_pool(name="ps", bufs=2, space="PSUM"))
    sb = ctx.enter_context(tc.tile_pool(name="sb", bufs=3))

    # ---- t0 loads: small critical-path DMAs on sync, weights on scalar ----
    nc.sync.dma_start(idx_sb[:].rearrange("p (c o) -> p c o", o=1),
                      tok_idx.rearrange("(c p) o -> p c o", p=P))
    # feature rows land directly in transposed layout (host pre-transposed)
    nc.sync.dma_start(
        xT[:].rearrange("p (k r) -> p k r", k=NK)[:, :, 0:BL],
        x_featT.rearrange("(k p) b -> p k b", p=P))
    nc.sync.dma_start(biast_sb[:], bias_t[:])
    nc.scalar.dma_start(wih_sb[:].rearrange("p (k f) -> p k f", k=NK),
                        w_ihT.rearrange("(k p) f -> p k f", p=P))

    # ---- embedding gather, first thing on the gpsimd queue ----
    g1 = nc.gpsimd.indirect_dma_start(
        out=xn0[:], out_offset=None, in_=embed_w[:],
        in_offset=bass.IndirectOffsetOnAxis(ap=idx_sb[:, 0:1], axis=0))
    g2 = nc.gpsimd.indirect_dma_start(
        out=xn1[:], out_offset=None, in_=embed_w[:],
        in_offset=bass.IndirectOffsetOnAxis(ap=idx_sb[:, 1:2], axis=0))

    make_identity(nc, ident[:])
    # zero the pad columns so phase-A matmuls can't stream NaN bit patterns
    nc.gpsimd.memset(
        xT[:].rearrange("p (k r) -> p k r", k=NK)[:, :, BL:FPAD], 0)
    nc.scalar.dma_start(whh_sb[:].rearrange("p (k f) -> p k f", k=NK),
                        w_hhT.rearrange("(k p) f -> p k f", p=P))

    # FC weights stream on the scalar queue, gated on the gather so they
    # can't starve the head DMAs.
    for k in range(NK):
        fdma = nc.scalar.dma_start(fcw_sb[:, k * VP:(k + 1) * VP],
                                   fc_wT[k * P:(k + 1) * P, :])
        if k == 0:
            _adh(fdma.ins, g2.ins, sync=True,
                 reason="delay fc weight stream past head")
    nc.scalar.dma_start(fcb_sb[:], fc_b_bc[:])

    # ---- PE warm-up: junk matmuls gated on the gather so the HAM
    # clock-gate opens right before the real PE work ----
    wps = ps.tile([P, 512], f32, name="wps", tag="ps")
    last_warm = None
    for wi in range(72):
        wmm = nc.tensor.matmul(wps[:, :P], lhsT=ident[:], rhs=ident[:],
                               start=True, stop=True)
        if wi == 0:
            _adh(wmm.ins, g1.ins, sync=False, reason="warmup after gather")
        last_warm = wmm

    # ---- transpose gathered rows -> X.T columns. xn0 goes through the
    # XBAR on the sync queue (zero PE); xn1 through the (otherwise idle)
    # PE so neither serializes behind the other. ----
    _XTAGS = ("gg", "gif")
    for k in range(NK):
        nc.sync.dma_start(xT[:, k * ROWS + FPAD: k * ROWS + FPAD + P],
                          xn0[:, k * P:(k + 1) * P], transpose=True)
    for k in range(NK):
        pt = ps.tile([P, 512], bf16, name="pst", tag=_XTAGS[k % 2])
        nc.tensor.transpose(pt[:, :P], xn1[:, k * P:(k + 1) * P], ident[:])
        nc.vector.tensor_copy(
            out=xT[:, k * ROWS + FPAD + P: k * ROWS + FPAD + 2 * P],
            in_=pt[:, :P])

    # ---- X projection GEMM:  xpT[:, c*128 + j*8 + b] = (X @ W_ihT)[row, j*128+p]
    # Phase A covers features + steps 1..16 and runs before the recurrence;
    # phase B (steps 17..32) is emitted one j-group per early recurrence
    # step, filling the per-step PE stall. ----
    _TAGS = ("gg", "gif", "go", "ps")
    xp_view = xpT[:].rearrange("p (s j b) -> p s j b", s=STEPS, j=NJ, b=BL)
    for j in range(NJ):
        pxp = ps.tile([P, 512], f32, name="pxp", tag=_TAGS[j % 4])
        for k in range(NK):
            nc.tensor.matmul(
                pxp[:, :ARW],
                lhsT=wih_sb[:, k * FOURH + j * P: k * FOURH + (j + 1) * P],
                rhs=xT[:, k * ROWS: k * ROWS + ARW],
                start=(k == 0), stop=(k == NK - 1))
        # cols [0:8] features -> cell 0; [16:144] tokens t=0..15 -> cells 1..16
        nc.vector.tensor_scalar_add(
            xp_view[:, 0, j, :], pxp[:, 0:BL], biast_sb[:, j:j + 1])
        nc.vector.tensor_scalar_add(
            xp_view[:, 1:17, j, :],
            pxp[:, FPAD:ARW].rearrange("p (s b) -> p s b", b=BL),
            biast_sb[:, j:j + 1])

    def _xproj_b(j):
        pxb = ps.tile([P, 512], f32, name="pxb", tag="ps")
        for k in range(NK):
            nc.tensor.matmul(
                pxb[:, :TOKR // 2],
                lhsT=wih_sb[:, k * FOURH + j * P: k * FOURH + (j + 1) * P],
                rhs=xT[:, k * ROWS + ARW: (k + 1) * ROWS],
                start=(k == 0), stop=(k == NK - 1))
        nc.vector.tensor_scalar_add(
            xp_view[:, 17:STEPS, j, :],
            pxb[:, :TOKR // 2].rearrange("p (s b) -> p s b", b=BL),
            biast_sb[:, j:j + 1])

    # ---- FC work-chunk emitter ----
    # One chunk = one 512-wide vocab slice for one 128-row token block:
    # 4 PE matmuls + a DVE bias-add into a bf16 staging tile + output DMA.
    out_v = out[:, :, :]   # [BL, T, V]
    _fc_n = [0]

    def _fc_chunk(m, n, tag="ps"):
        pfc = ps.tile([P, 512], f32, name="pfc", tag=tag)
        lhs_off = m * P
        for k in range(NK):
            nc.tensor.matmul(
                pfc,
                lhsT=hT[:, k * TOKR + lhs_off: k * TOKR + lhs_off + P],
                rhs=fcw_sb[:, k * VP + n * 512: k * VP + (n + 1) * 512],
                start=(k == 0), stop=(k == NK - 1))
        stg = sb.tile([P, 512], bf16, name="stg", tag="stg")
        nc.vector.tensor_add(out=stg[:], in0=pfc[:],
                             in1=fcb_sb[:, n * 512:(n + 1) * 512])
        glo = n * 512
        gw = min(V, glo + 512) - glo
        if gw <= 0:
            return
        eng = nc.sync if _fc_n[0] % 2 == 0 else nc.scalar
        _fc_n[0] += 1
        eng.dma_start(
            out=out_v[:, m * 16:(m + 1) * 16, glo:glo + gw]
            .rearrange("b t v -> t b v"),
            in_=stg[:, :gw])

    # m=0 token block (steps 1..16) interleaves into the recurrence tail;
    # 20 chunks over steps 17..31 (2 on every third step).
    fc_sched = {}
    chunks = [(0, n) for n in range(NV)]
    ci = 0
    for c in range(17, 32):
        take = 2 if (c - 17) % 3 == 0 else 1
        fc_sched[c] = chunks[ci:ci + take]
        ci += take
    assert ci == NV

    # ---- recurrence ----
    hT_view = hT[:].rearrange("p (k s b) -> p k s b", k=NK, s=T, b=BL)
    h0_view = h0T[:].rearrange("p (k b) -> p k b", k=NK)

    def _hprev(c, k):
        if c == 1:
            return h0T[:, k * BL:(k + 1) * BL]
        off = k * TOKR + (c - 2) * BL
        return hT[:, off: off + BL]

    # Gate groups: (name, j-range, xp column offset, width) in (if, g, o) order
    GRP = (("gif", 0, 8, 0, 64), ("gg", 8, 12, 64, 32), ("go", 12, 16, 96, 32))

    for c in range(STEPS):
        if c == 0:
            g_if, g_g, g_o = (xpT[:, 0:64], xpT[:, 64:96], xpT[:, 96:128])
        else:
            tiles = {}
            for (tag, j0, j1, xoff, wdt) in GRP:
                pg = ps.tile([P, 64], f32, name=tag, tag=tag)[:, :wdt]
                tiles[tag] = pg
                # identity matmul preloads PSUM with the X-projection term
                # (start=True sets has_written so W matmuls accumulate)
                nc.tensor.matmul(
                    pg, lhsT=ident[:], rhs=xpT[:, c * P + xoff: c * P + xoff + wdt],
                    start=True, stop=False, skip_group_check=True)
                for k in range(NK):
                    for j in range(j0, j1):
                        nc.tensor.matmul(
                            pg[:, (j - j0) * BL:(j - j0 + 1) * BL],
                            lhsT=whh_sb[:, k * FOURH + j * P: k * FOURH + (j + 1) * P],
                            rhs=_hprev(c, k),
                            start=False, stop=(j == j1 - 1 and k == NK - 1),
                            skip_group_check=True)
            g_g, g_if, g_o = tiles["gg"], tiles["gif"], tiles["go"]

        act_g = sb.tile([P, 32], f32, name="act_g")
        act_if = sb.tile([P, 64], f32, name="act_if")
        act_o = sb.tile([P, 32], f32, name="act_o")
        nc.scalar.activation(act_if[:], g_if,
                             mybir.ActivationFunctionType.Sigmoid)
        # f*c_prev only needs the i/f sigmoid: run it on DVE while the
        # g-tanh is still streaming on the scalar engine.
        fc2 = sb.tile([P, 32], f32, name="fc2")
        if c > 0:
            nc.vector.tensor_mul(out=fc2[:], in0=act_if[:, 32:64], in1=cst[:])
        nc.scalar.activation(act_g[:], g_g,
                             mybir.ActivationFunctionType.Tanh)
        nc.scalar.activation(act_o[:], g_o,
                             mybir.ActivationFunctionType.Sigmoid)

        if c == 0:
            hdst = h0_view
        else:
            hdst = hT_view[:, :, c - 1, :]
        o_v = act_o[:].rearrange("p (k b) -> p k b", k=NK)
        ig = sb.tile([P, 32], f32, name="ig")
        tch = sb.tile([P, 32], f32, name="tch")
        # c/h update in k-halves so the next step's k=0/1 matmuls can
        # start while the second half is still in the DVE/ACT chain
        for lo, hi in ((0, 2), (2, 4)):
            sl = slice(lo * BL, hi * BL)
            if c == 0:
                # c_new = i * g  (previous c is zero)
                nc.vector.tensor_mul(out=cst[:, sl], in0=act_if[:, sl],
                                     in1=act_g[:, sl])
            else:
                nc.vector.tensor_mul(out=ig[:, sl], in0=act_if[:, sl],
                                     in1=act_g[:, sl])
                nc.vector.tensor_add(out=cst[:, sl], in0=ig[:, sl],
                                     in1=fc2[:, sl])
            nc.scalar.activation(tch[:, sl], cst[:, sl],
                                 mybir.ActivationFunctionType.Tanh)
            nc.vector.tensor_mul(
                out=hdst[:, lo:hi, :], in0=o_v[:, lo:hi, :],
                in1=tch[:, sl].rearrange("p (k b) -> p k b", b=BL))

        if 1 <= c <= NJ:
            _xproj_b(c - 1)
        for (m, n) in fc_sched.get(c, ()):
            _fc_chunk(m, n)

    # ---- remaining FC chunks (m=1 token block); the gate PSUM tags are
    # free after the last step, so rotate all four for deeper pipelining ----
    for n in range(NV):
        _fc_chunk(1, n, tag=_TAGS[n % 4])
    ctx.close()


_NC_CACHE = {}


def _get_program():
    if "nc" not in _NC_CACHE:
        _NC_CACHE["nc"] = _build_program()
    return _NC_CACHE["nc"]


def make_in_maps(features, captions, embed_W, W_ih, W_hh, b_ih, b_hh, fc_W, fc_b):
    """Host-side sharding + layout prep. Pure layout/dtype work, no math
    beyond summing the two bias vectors."""
    embed_bf = embed_W.astype(BF16)
    w_ihT = np.ascontiguousarray(W_ih.T[:, _PERM]).astype(BF16)
    w_hhT = np.ascontiguousarray(W_hh.T[:, _PERM]).astype(BF16)
    bias = (b_ih + b_hh).astype(np.float32)[_PERM]
    bias_t = np.ascontiguousarray(bias.reshape(NJ, P).T)
    fc_wT = np.zeros((H, VP), dtype=BF16)
    fc_wT[:, :V] = fc_W.T.astype(BF16)
    fcb = np.zeros((VP,), dtype=BF16)
    fcb[:V] = fc_b.astype(BF16)
    fc_b_bc = np.ascontiguousarray(np.broadcast_to(fcb, (P, VP)))

    in_maps = []
    for core in range(NCORES):
        sl = slice(core * BL, (core + 1) * BL)
        cap = captions[sl].astype(np.int32)          # [BL, T]
        tok = np.ascontiguousarray(cap.T).reshape(TOKR, 1)  # t-major
        in_maps.append({
            "x_featT": np.ascontiguousarray(features[sl].T).astype(BF16),
            "tok_idx": tok,
            "embed_w": embed_bf,
            "w_ihT": w_ihT,
            "w_hhT": w_hhT,
            "bias_t": bias_t,
            "fc_wT": fc_wT,
            "fc_b_bc": fc_b_bc,
        })
    return in_maps


def _ensure_ntff_hook():
    """The agent image's antenv package lacks axon_hooks; synthesize it so
    run_bass_kernel_spmd(trace=True) can capture NTFF profiles."""
    import sys
    import types
    try:
        from antenv.axon_hooks import get_axon_ntff_profile_hook  # noqa: F401
        return
    except ImportError:
        pass
    import antenv
    mod = types.ModuleType("antenv.axon_hooks")
    state = {}
    mod.set_axon_ntff_profile_hook = lambda h: state.__setitem__("h", h)
    mod.get_axon_ntff_profile_hook = lambda: state.get("h")
    sys.modules["antenv.axon_hooks"] = mod
    antenv.axon_hooks = mod
    try:
        from trn_agent_boot.trn_boot import _ntff_profile_via_ctypes
        hook = _ntff_profile_via_ctypes("/opt/axon/libaxon_pjrt.so")
        if hook is not None:
            mod.set_axon_ntff_profile_hook(hook)
    except Exception as e:  # degrade: tracing skipped, run still works
        print(f"ntff hook setup failed: {e}")


def kernel(features, captions, embed_W, W_ih, W_hh, b_ih, b_hh, fc_W, fc_b,
           _trace=False):
    nc = _get_program()
    in_maps = make_in_maps(features, captions, embed_W, W_ih, W_hh,
                           b_ih, b_hh, fc_W, fc_b)
    if _trace:
        _ensure_ntff_hook()
    res = bass_utils.run_bass_kernel_spmd(
        nc, in_maps, core_ids=list(range(NCORES)), trace=_trace)
    out = np.concatenate(
        [res.results[c]["out"].astype(np.float32) for c in range(NCORES)],
        axis=0)
    if _trace:
        kernel.last_result = res
    return out


# revision 43
# speedup vs baseline: 1.0020x; 1.0020x over previous
"""DecoderRNN (LSTM + vocab projection) Trainium2 kernel.

Strategy: data-parallel over batch B=64 across 8 NeuronCores (8 examples
per core). Per core:
  1. indirect-DMA gather of caption embeddings (bf16) issued at t0,
     XBAR DMA-transpose -> X.T (no PE involvement); feature rows arrive
     pre-transposed from the host.
  2. one GEMM precomputes X @ W_ih.T + b for all 33 cell steps,
     rotating across 4 PSUM banks so the PE never stalls on drain.
  3. 33 sequential LSTM cell steps in transposed layout:
       gates.T = W_hh.T.T @ h.T accumulated into PSUM tiles split by gate
       group (i+f / g / o), each preloaded with its X-projection term via an
       identity matmul; eltwise runs as a few [128, 32..64] ACT/DVE ops.
       From step 17 on, FC chunks for the first half of the tokens are
       interleaved between steps to fill the per-step PE stall and to
       spread the output writeback across the recurrence window.
  4. remaining FC GEMM chunks + bias, bf16 output DMA (host upcasts).
All matmuls take bf16 inputs with fp32 PSUM accumulation.
"""

import numpy as np
import ml_dtypes

import concourse.bass as bass
import concourse.tile as tile
from concourse import bacc, mybir
from concourse import bass_utils
from concourse.masks import make_identity

BF16 = ml_dtypes.bfloat16

# Problem shape (hardcoded per the task contract).
B, T, E, H, V = 64, 32, 512, 512, 10000
NCORES = 8
BL = B // NCORES            # 8 examples per core
STEPS = T + 1               # 33 cell steps (features + 32 caption tokens)
FOURH = 4 * H               # 2048
P = 128
NJ = FOURH // P             # 16 gate-unit chunks
NK = H // P                 # 4 contraction chunks
TOKR = T * BL               # 256 token rows (t-major)
FPAD = 2 * BL               # features (8) + pad (8): 16-aligned prefix so
                            # XBAR transpose dest offsets stay tile-aligned
ROWS = FPAD + TOKR          # 272 X.T columns per k-chunk:
                            # [features 0:8 | pad 8:16 | tokens 16:272]
ARW = FPAD + TOKR // 2      # 144 = xproj phase-A columns (features+steps 1-16)
VP = 10240                  # padded vocab (20 * 512)
NV = VP // 512              # 20 vocab chunks

f32 = mybir.dt.float32
bf16 = mybir.dt.bfloat16
i32 = mybir.dt.int32
fp8 = mybir.dt.float8e4
FP8 = ml_dtypes.float8_e4m3

# Gate pack order along 4H is (i, f, g, o) — PyTorch's native order, so the
# chain-critical i/f sigmoids can start while the g/o matmuls still stream.
_PERM = np.arange(4 * H)


def _build_program():
    nc = bacc.Bacc(
        "TRN2",
        target_bir_lowering=False,
        debug=False,
        num_devices=NCORES,
    )

    x_featT = nc.dram_tensor("x_featT", [E, BL], bf16, kind="ExternalInput").ap()
    tok_idx = nc.dram_tensor("tok_idx", [TOKR, 1], i32, kind="ExternalInput").ap()
    embed_w = nc.dram_tensor("embed_w", [V, E], bf16, kind="ExternalInput").ap()
    w_ihT = nc.dram_tensor("w_ihT", [E, FOURH], bf16, kind="ExternalInput").ap()
    w_hhT = nc.dram_tensor("w_hhT", [H, FOURH], bf16, kind="ExternalInput").ap()
    bias_t = nc.dram_tensor("bias_t", [P, NJ], f32, kind="ExternalInput").ap()
    fc_wT = nc.dram_tensor("fc_wT", [H, VP], bf16, kind="ExternalInput").ap()
    fc_b_bc = nc.dram_tensor("fc_b_bc", [P, VP], bf16, kind="ExternalInput").ap()
    out = nc.dram_tensor("out", [BL, T, V], bf16, kind="ExternalOutput").ap()

    with tile.TileContext(nc) as tc:
        _kernel_body(tc, x_featT, tok_idx, embed_w, w_ihT, w_hhT, bias_t,
                     fc_wT, fc_b_bc, out)

    nc.compile()
    return nc


def _kernel_body(tc, x_featT, tok_idx, embed_w, w_ihT, w_hhT, bias_t,
                 fc_wT, fc_b_bc, out):
    from contextlib import ExitStack
    from concourse.tile_rust import add_dep_helper as _adh
    ctx = ExitStack()
    nc = tc.nc

    # ---- persistent tiles (one bufs=1 pool, distinct tags per name) ----
    cp = ctx.enter_context(tc.tile_pool(name="const", bufs=1))
    wih_sb = cp.tile([P, NK * FOURH], bf16, name="wih_sb", tag="wih_sb")
    whh_sb = cp.tile([P, NK * FOURH], bf16, name="whh_sb", tag="whh_sb")
    fcw_sb = cp.tile([P, NK * VP], bf16, name="fcw_sb", tag="fcw_sb")
    fcb_sb = cp.tile([P, VP], bf16, name="fcb_sb", tag="fcb_sb")
    biast_sb = cp.tile([P, NJ], f32, name="biast_sb", tag="biast_sb")
    ident = cp.tile([P, P], bf16, name="ident", tag="ident")
    idx_sb = cp.tile([P, 2], i32, name="idx_sb", tag="idx_sb")
    xn0 = cp.tile([P, E], bf16, name="xn0", tag="xn0")
    xn1 = cp.tile([P, E], bf16, name="xn1", tag="xn1")
    xT = cp.tile([P, NK * ROWS], bf16, name="xT", tag="xT")
    xpT = cp.tile([P, STEPS * P], bf16, name="xpT", tag="xpT")
    hT = cp.tile([P, NK * TOKR], bf16, name="hT", tag="hT")
    h0T = cp.tile([P, NK * BL], bf16, name="h0T", tag="h0T")
    cst = cp.tile([P, NK * BL], f32, name="cst", tag="cst")

    ps = ctx.enter_context(tc.tile_pool(name="ps", bufs=2, space="PSUM"))
    sb = ctx.enter_context(tc.tile_pool(name="sb", bufs=3))

    # ---- t0 loads: small critical-path DMAs on sync, weights on scalar ----
    nc.sync.dma_start(idx_sb[:].rearrange("p (c o) -> p c o", o=1),
                      tok_idx.rearrange("(c p) o -> p c o", p=P))
    # feature rows land directly in transposed layout (host pre-transposed)
    nc.sync.dma_start(
        xT[:].rearrange("p (k r) -> p k r", k=NK)[:, :, 0:BL],
        x_featT.rearrange("(k p) b -> p k b", p=P))
    nc.sync.dma_start(biast_sb[:], bias_t[:])
    nc.scalar.dma_start(wih_sb[:].rearrange("p (k f) -> p k f", k=NK),
                        w_ihT.rearrange("(k p) f -> p k f", p=P))

    # ---- embedding gather, first thing on the gpsimd queue ----
    g1 = nc.gpsimd.indirect_dma_start(
        out=xn0[:], out_offset=None, in_=embed_w[:],
        in_offset=bass.IndirectOffsetOnAxis(ap=idx_sb[:, 0:1], axis=0))
    g2 = nc.gpsimd.indirect_dma_start(
        out=xn1[:], out_offset=None, in_=embed_w[:],
        in_offset=bass.IndirectOffsetOnAxis(ap=idx_sb[:, 1:2], axis=0))

    make_identity(nc, ident[:])
    # zero the pad columns so phase-A matmuls can't stream NaN bit patterns
    nc.gpsimd.memset(
        xT[:].rearrange("p (k r) -> p k r", k=NK)[:, :, BL:FPAD], 0)
    nc.scalar.dma_start(whh_sb[:].rearrange("p (k f) -> p k f", k=NK),
                        w_hhT.rearrange("(k p) f -> p k f", p=P))

    # FC weights stream on the scalar queue, gated on the gather so they
    # can't starve the head DMAs.
    for k in range(NK):
        fdma = nc.scalar.dma_start(fcw_sb[:, k * VP:(k + 1) * VP],
                                   fc_wT[k * P:(k + 1) * P, :])
        if k == 0:
            _adh(fdma.ins, g2.ins, sync=True,
                 reason="delay fc weight stream past head")
    nc.scalar.dma_start(fcb_sb[:], fc_b_bc[:])

    # ---- PE warm-up: junk matmuls gated on the gather so the HAM
    # clock-gate opens right before the real PE work ----
    wps = ps.tile([P, 512], f32, name="wps", tag="ps")
    last_warm = None
    for wi in range(72):
        wmm = nc.tensor.matmul(wps[:, :P], lhsT=ident[:], rhs=ident[:],
                               start=True, stop=True)
        if wi == 0:
            _adh(wmm.ins, g1.ins, sync=False, reason="warmup after gather")
        last_warm = wmm

    # ---- transpose gathered rows -> X.T columns. xn0 goes through the
    # XBAR on the sync queue (zero PE); xn1 through the (otherwise idle)
    # PE so neither serializes behind the other. ----
    _XTAGS = ("gg", "gif")
    for k in range(NK):
        nc.sync.dma_start(xT[:, k * ROWS + FPAD: k * ROWS + FPAD + P],
                          xn0[:, k * P:(k + 1) * P], transpose=True)
    for k in range(NK):
        pt = ps.tile([P, 512], bf16, name="pst", tag=_XTAGS[k % 2])
        nc.tensor.transpose(pt[:, :P], xn1[:, k * P:(k + 1) * P], ident[:])
        nc.vector.tensor_copy(
            out=xT[:, k * ROWS + FPAD + P: k * ROWS + FPAD + 2 * P],
            in_=pt[:, :P])

    # ---- X projection GEMM:  xpT[:, c*128 + j*8 + b] = (X @ W_ihT)[row, j*128+p]
    # Phase A covers features + steps 1..16 and runs before the recurrence;
    # phase B (steps 17..32) is emitted one j-group per early recurrence
    # step, filling the per-step PE stall. ----
    _TAGS = ("gg", "gif", "go", "ps")
    xp_view = xpT[:].rearrange("p (s j b) -> p s j b", s=STEPS, j=NJ, b=BL)
    for j in range(NJ):
        pxp = ps.tile([P, 512], f32, name="pxp", tag=_TAGS[j % 4])
        for k in range(NK):
            nc.tensor.matmul(
                pxp[:, :ARW],
                lhsT=wih_sb[:, k * FOURH + j * P: k * FOURH + (j + 1) * P],
                rhs=xT[:, k * ROWS: k * ROWS + ARW],
                start=(k == 0), stop=(k == NK - 1))
        # cols [0:8] features -> cell 0; [16:144] tokens t=0..15 -> cells 1..16
        nc.vector.tensor_scalar_add(
            xp_view[:, 0, j, :], pxp[:, 0:BL], biast_sb[:, j:j + 1])
        nc.vector.tensor_scalar_add(
            xp_view[:, 1:17, j, :],
            pxp[:, FPAD:ARW].rearrange("p (s b) -> p s b", b=BL),
            biast_sb[:, j:j + 1])

    def _xproj_b(j):
        pxb = ps.tile([P, 512], f32, name="pxb", tag="ps")
        for k in range(NK):
            nc.tensor.matmul(
                pxb[:, :TOKR // 2],
                lhsT=wih_sb[:, k * FOURH + j * P: k * FOURH + (j + 1) * P],
                rhs=xT[:, k * ROWS + ARW: (k + 1) * ROWS],
                start=(k == 0), stop=(k == NK - 1))
        nc.vector.tensor_scalar_add(
            xp_view[:, 17:STEPS, j, :],
            pxb[:, :TOKR // 2].rearrange("p (s b) -> p s b", b=BL),
            biast_sb[:, j:j + 1])

    # ---- FC work-chunk emitter ----
    # One chunk = one 512-wide vocab slice for one 128-row token block:
    # 4 PE matmuls + a DVE bias-add into a bf16 staging tile + output DMA.
    out_v = out[:, :, :]   # [BL, T, V]
    _fc_n = [0]

    def _fc_chunk(m, n, tag="ps"):
        pfc = ps.tile([P, 512], f32, name="pfc", tag=tag)
        lhs_off = m * P
        for k in range(NK):
            nc.tensor.matmul(
                pfc,
                lhsT=hT[:, k * TOKR + lhs_off: k * TOKR + lhs_off + P],
                rhs=fcw_sb[:, k * VP + n * 512: k * VP + (n + 1) * 512],
                start=(k == 0), stop=(k == NK - 1))
        stg = sb.tile([P, 512], bf16, name="stg", tag="stg")
        nc.vector.tensor_add(out=stg[:], in0=pfc[:],
                             in1=fcb_sb[:, n * 512:(n + 1) * 512])
        glo = n * 512
        gw = min(V, glo + 512) - glo
        if gw <= 0:
            return
        eng = nc.sync if _fc_n[0] % 2 == 0 else nc.scalar
        _fc_n[0] += 1
        eng.dma_start(
            out=out_v[:, m * 16:(m + 1) * 16, glo:glo + gw]
            .rearrange("b t v -> t b v"),
            in_=stg[:, :gw])

    # m=0 token block (steps 1..16) interleaves into the recurrence tail;
    # 20 chunks over steps 17..31 (2 on every third step).
    fc_sched = {}
    chunks = [(0, n) for n in range(NV)]
    ci = 0
    for c in range(17, 32):
        take = 2 if (c - 17) % 3 == 0 else 1
        fc_sched[c] = chunks[ci:ci + take]
        ci += take
    assert ci == NV

    # ---- recurrence ----
    hT_view = hT[:].rearrange("p (k s b) -> p k s b", k=NK, s=T, b=BL)
    h0_view = h0T[:].rearrange("p (k b) -> p k b", k=NK)

    def _hprev(c, k):
        if c == 1:
            return h0T[:, k * BL:(k + 1) * BL]
        off = k * TOKR + (c - 2) * BL
        return hT[:, off: off + BL]

    # Gate groups: (name, j-range, xp column offset, width) in (if, g, o) order
    GRP = (("gif", 0, 8, 0, 64), ("gg", 8, 12, 64, 32), ("go", 12, 16, 96, 32))

    for c in range(STEPS):
        if c == 0:
            g_if, g_g, g_o = (xpT[:, 0:64], xpT[:, 64:96], xpT[:, 96:128])
        else:
            tiles = {}
            for (tag, j0, j1, xoff, wdt) in GRP:
                pg = ps.tile([P, 64], f32, name=tag, tag=tag)[:, :wdt]
                tiles[tag] = pg
                # identity matmul preloads PSUM with the X-projection term
                # (start=True sets has_written so W matmuls accumulate)
                nc.tensor.matmul(
                    pg, lhsT=ident[:], rhs=xpT[:, c * P + xoff: c * P + xoff + wdt],
                    start=True, stop=False, skip_group_check=True)
                for k in range(NK):
                    for j in range(j0, j1):
                        nc.tensor.matmul(
                            pg[:, (j - j0) * BL:(j - j0 + 1) * BL],
                            lhsT=whh_sb[:, k * FOURH + j * P: k * FOURH + (j + 1) * P],
                            rhs=_hprev(c, k),
                            start=False, stop=(j == j1 - 1 and k == NK - 1),
                            skip_group_check=True)
            g_g, g_if, g_o = tiles["gg"], tiles["gif"], tiles["go"]

        act_g = sb.tile([P, 32], f32, name="act_g")
        act_if = sb.tile([P, 64], f32, name="act_if")
        act_o = sb.tile([P, 32], f32, name="act_o")
        nc.scalar.activation(act_if[:], g_if,
                             mybir.ActivationFunctionType.Sigmoid)
        # f*c_prev only needs the i/f sigmoid: run it on DVE while the
        # g-tanh is still streaming on the scalar engine.
        fc2 = sb.tile([P, 32], f32, name="fc2")
        if c > 0:
            nc.vector.tensor_mul(out=fc2[:], in0=act_if[:, 32:64], in1=cst[:])
        nc.scalar.activation(act_g[:], g_g,
                             mybir.ActivationFunctionType.Tanh)
        nc.scalar.activation(act_o[:], g_o,
                             mybir.ActivationFunctionType.Sigmoid)

        if c == 0:
            # c_new = i * g  (previous c is zero)
            nc.vector.tensor_mul(out=cst[:], in0=act_if[:, 0:32], in1=act_g[:])
        else:
            ig = sb.tile([P, 32], f32, name="ig")
            nc.vector.tensor_mul(out=ig[:], in0=act_if[:, 0:32], in1=act_g[:])
            nc.vector.tensor_add(out=cst[:], in0=ig[:], in1=fc2[:])

        tch = sb.tile([P, 32], f32, name="tch")
        nc.scalar.activation(tch[:], cst[:], mybir.ActivationFunctionType.Tanh)

        if c == 0:
            hdst = h0_view
        else:
            hdst = hT_view[:, :, c - 1, :]
        o_v = act_o[:].rearrange("p (k b) -> p k b", k=NK)
        t_v = tch[:].rearrange("p (k b) -> p k b", k=NK)
        nc.vector.tensor_mul(out=hdst[:, 0:2, :], in0=o_v[:, 0:2, :],
                             in1=t_v[:, 0:2, :])
        nc.vector.tensor_mul(out=hdst[:, 2:4, :], in0=o_v[:, 2:4, :],
                             in1=t_v[:, 2:4, :])

        if 1 <= c <= NJ:
            _xproj_b(c - 1)
        for (m, n) in fc_sched.get(c, ()):
            _fc_chunk(m, n)

    # ---- remaining FC chunks (m=1 token block); the gate PSUM tags are
    # free after the last step, so rotate all four for deeper pipelining ----
    for n in range(NV):
        _fc_chunk(1, n, tag=_TAGS[n % 4])
    ctx.close()


_NC_CACHE = {}


def _get_program():
    if "nc" not in _NC_CACHE:
        _NC_CACHE["nc"] = _build_program()
    return _NC_CACHE["nc"]


def make_in_maps(features, captions, embed_W, W_ih, W_hh, b_ih, b_hh, fc_W, fc_b):
    """Host-side sharding + layout prep. Pure layout/dtype work, no math
    beyond summing the two bias vectors."""
    embed_bf = embed_W.astype(BF16)
    w_ihT = np.ascontiguousarray(W_ih.T[:, _PERM]).astype(BF16)
    w_hhT = np.ascontiguousarray(W_hh.T[:, _PERM]).astype(BF16)
    bias = (b_ih + b_hh).astype(np.float32)[_PERM]
    bias_t = np.ascontiguousarray(bias.reshape(NJ, P).T)
    fc_wT = np.zeros((H, VP), dtype=BF16)
    fc_wT[:, :V] = fc_W.T.astype(BF16)
    fcb = np.zeros((VP,), dtype=BF16)
    fcb[:V] = fc_b.astype(BF16)
    fc_b_bc = np.ascontiguousarray(np.broadcast_to(fcb, (P, VP)))

    in_maps = []
    for core in range(NCORES):
        sl = slice(core * BL, (core + 1) * BL)
        cap = captions[sl].astype(np.int32)          # [BL, T]
        tok = np.ascontiguousarray(cap.T).reshape(TOKR, 1)  # t-major
        in_maps.append({
            "x_featT": np.ascontiguousarray(features[sl].T).astype(BF16),
            "tok_idx": tok,
            "embed_w": embed_bf,
            "w_ihT": w_ihT,
            "w_hhT": w_hhT,
            "bias_t": bias_t,
            "fc_wT": fc_wT,
            "fc_b_bc": fc_b_bc,
        })
    return in_maps


def _ensure_ntff_hook():
    """The agent image's antenv package lacks axon_hooks; synthesize it so
    run_bass_kernel_spmd(trace=True) can capture NTFF profiles."""
    import sys
    import types
    try:
        from antenv.axon_hooks import get_axon_ntff_profile_hook  # noqa: F401
        return
    except ImportError:
        pass
    import antenv
    mod = types.ModuleType("antenv.axon_hooks")
    state = {}
    mod.set_axon_ntff_profile_hook = lambda h: state.__setitem__("h", h)
    mod.get_axon_ntff_profile_hook = lambda: state.get("h")
    sys.modules["antenv.axon_hooks"] = mod
    antenv.axon_hooks = mod
    try:
        from trn_agent_boot.trn_boot import _ntff_profile_via_ctypes
        hook = _ntff_profile_via_ctypes("/opt/axon/libaxon_pjrt.so")
        if hook is not None:
            mod.set_axon_ntff_profile_hook(hook)
    except Exception as e:  # degrade: tracing skipped, run still works
        print(f"ntff hook setup failed: {e}")


def kernel(features, captions, embed_W, W_ih, W_hh, b_ih, b_hh, fc_W, fc_b,
           _trace=False):
    nc = _get_program()
    in_maps = make_in_maps(features, captions, embed_W, W_ih, W_hh,
                           b_ih, b_hh, fc_W, fc_b)
    if _trace:
        _ensure_ntff_hook()
    res = bass_utils.run_bass_kernel_spmd(
        nc, in_maps, core_ids=list(range(NCORES)), trace=_trace)
    out = np.concatenate(
        [res.results[c]["out"].astype(np.float32) for c in range(NCORES)],
        axis=0)
    if _trace:
        kernel.last_result = res
    return out
